# revision 8
# baseline (speedup 1.0000x reference)
"""Single-head causal attention (B=8, T=2048, E=1024, H=64) on 8 NeuronCores.

Sharding: data-parallel over batch — one batch element per core.

Per-core algorithm (matmul inputs in DT = fp32r or bf16, fp32 accumulate):
  inputs (host-prepped): xT = x[b].T  [E, T],  wqk = [Wq | Wk]  [E, 128],
                         wv = Wv      [E, H]
  1. QK^T projection:  psum[0:64]  = Q^T = Wq^T xT   [64, T]
                       psum[64:128]= K^T = Wk^T xT   (packed stationary [E,128])
     V^T projection:   V^T = Wv^T xT                 [64, T]
  2. V^T is PE-transposed back to natural V [T, 64] tiles, with a ones
     column appended (V' = [V | 1]) so the PV matmul also produces the
     softmax denominator l_q = sum_k exp(s_kq).
  3. Per 512-wide q-chunk, for each 128-wide k-tile (causal: k <= q only):
       S^T block = K Q^T   (lhsT = K^T k-slice, rhs = Q^T q-slice)
       P^T = exp(S^T * E**-0.5)   (no max subtraction: |scores| <= ~1.5)
       diagonal blocks masked with an upper-triangular 0/1 mask
       O'^T[65, q] += V'^T P^T    (row 64 accumulates l_q)
  4. O'^T chunks are PE-transposed to natural layout, divided by l_q
     (per-partition scalar), and DMA'd out as [T, H].
"""
import numpy as np
import ml_dtypes

import concourse.bass as bass
import concourse.bacc as bacc
import concourse.mybir as mybir
import concourse.tile as tile
from concourse.bass_utils import run_bass_kernel_spmd
from concourse.masks import make_identity, make_upper_triangular

B, T, E, H = 8, 2048, 1024, 64
P = 128
EO = E // P          # 8 e-tiles (contraction)
CH = 512             # q-chunk width
NCH = T // CH        # 4 chunks
KPC = CH // P        # 4 k-tiles per chunk
NKT = T // P         # 16 k-tiles total
SCALE = float(E) ** -0.5
F32 = mybir.dt.float32
N_CORES = 8

MM_DTYPE = mybir.dt.float32r  # matmul input dtype: float32r or bfloat16


def build_nc(dt=None):
    dt = dt or MM_DTYPE
    nc = bacc.Bacc("TRN2", target_bir_lowering=False, debug=False)

    xt = nc.declare_dram_parameter("xt", [E, T], dt, isOutput=False)
    wqk = nc.declare_dram_parameter("wqk", [E, 2 * H], dt, isOutput=False)
    wv = nc.declare_dram_parameter("wv", [E, H], dt, isOutput=False)
    out = nc.declare_dram_parameter("out", [T, H], F32, isOutput=True)

    xt_r = xt.rearrange("(eo p) t -> eo p t", p=P)      # [8, 128, T]
    wqk_r = wqk.rearrange("(eo p) c -> p eo c", p=P)    # [128, 8, 128]
    wv_r = wv.rearrange("(eo p) c -> p eo c", p=P)      # [128, 8, 64]
    out_r = out.rearrange("(t p) h -> p t h", p=P)      # [128, 16, 64]

    with tile.TileContext(nc) as tc:
        with tc.tile_pool(name="const", bufs=1) as const, \
             tc.tile_pool(name="xq", bufs=4) as xpool, \
             tc.tile_pool(name="big", bufs=1) as big, \
             tc.tile_pool(name="vq", bufs=2) as vqpool, \
             tc.tile_pool(name="pblk", bufs=4) as ppool, \
             tc.tile_pool(name="ovt", bufs=2) as ovpool, \
             tc.tile_pool(name="ps_qk", bufs=2, space="PSUM") as ps_qk, \
             tc.tile_pool(name="ps_v", bufs=1, space="PSUM") as ps_v, \
             tc.tile_pool(name="ps_st", bufs=2, space="PSUM") as ps_st, \
             tc.tile_pool(name="ps_pv", bufs=1, space="PSUM") as ps_pv, \
             tc.tile_pool(name="ps_tr", bufs=1, space="PSUM") as ps_tr:

            # ---- constants ----
            wqk_sb = const.tile([P, EO, 2 * H], dt)
            wv_sb = const.tile([P, EO, H], dt)
            nc.sync.dma_start(wqk_sb[:], wqk_r[:])
            nc.sync.dma_start(wv_sb[:], wv_r[:])
            ident = const.tile([P, P], F32)
            make_identity(nc, ident[:])
            identm = const.tile([P, P], dt)
            make_identity(nc, identm[:])
            tri = const.tile([P, P], dt)  # tri[r, c] = 1 if r <= c else 0
            make_upper_triangular(nc, tri[:], val=1.0, diag=True)

            # ---- persistent buffers ----
            qkT = big.tile([P, T], dt)        # rows 0:64 = Q^T, 64:128 = K^T
            kT0 = big.tile([H, T], dt)        # K^T moved to partitions 0:63
            vnat = big.tile([P, NKT, H + 1], dt)   # V' tiles, col 64 = 1.0
            onat = big.tile([P, NKT, H + 1], F32)  # O tiles + l column
            nc.vector.memset(vnat[:], 1.0)

            for tq in range(NCH):
                qs = slice(tq * CH, (tq + 1) * CH)

                # ---- load x^T quarter: 8 tiles [128, 512] ----
                xq = [xpool.tile([P, CH], dt, tag=f"xq{eo}", name=f"xq{eo}_{tq}")
                      for eo in range(EO)]
                for eo in range(EO):
                    nc.sync.dma_start(xq[eo][:], xt_r[eo, :, qs])

                # ---- QK projection (accumulate over E) ----
                qk_ps = ps_qk.tile([P, CH], F32)
                for eo in range(EO):
                    nc.tensor.matmul(qk_ps[:], wqk_sb[:, eo, :], xq[eo][:],
                                     start=(eo == 0), stop=(eo == EO - 1))
                nc.vector.tensor_copy(qkT[:, qs], qk_ps[:])
                nc.vector.tensor_copy(kT0[:, qs], qkT[64:128, qs])

                # ---- V projection ----
                v_ps = ps_v.tile([H, CH], F32)
                for eo in range(EO):
                    nc.tensor.matmul(v_ps[:], wv_sb[:, eo, :], xq[eo][:],
                                     start=(eo == 0), stop=(eo == EO - 1))
                vq_sb = vqpool.tile([H, CH], dt)
                nc.scalar.copy(vq_sb[:], v_ps[:])

                # ---- transpose V quarter into natural layout ----
                for j in range(KPC):
                    kt = tq * KPC + j
                    tr_ps = ps_tr.tile([P, H + 1], dt, tag="trv")
                    nc.tensor.transpose(tr_ps[:, 0:H], vq_sb[:, j * P:(j + 1) * P],
                                        identm[0:H, 0:H])
                    nc.vector.tensor_copy(vnat[:, kt, 0:H], tr_ps[:, 0:H])

                # ---- attention for q-chunk tq ----
                pv_ps = ps_pv.tile([H + 1, CH], F32)
                n_kt = (tq + 1) * KPC
                for i in range(n_kt):
                    diag_j = i - tq * KPC  # >= 0 on diagonal k-tiles
                    w0 = diag_j * P if diag_j >= 0 else 0
                    st_ps = ps_st.tile([P, CH], F32)
                    nc.tensor.matmul(
                        st_ps[:, w0:CH],
                        kT0[:, i * P:(i + 1) * P],
                        qkT[0:64, tq * CH + w0:(tq + 1) * CH],
                        start=True, stop=True)
                    p_sb = ppool.tile([P, CH], dt)
                    nc.scalar.activation(p_sb[:, w0:CH], st_ps[:, w0:CH],
                                         mybir.ActivationFunctionType.Exp,
                                         bias=0.0, scale=SCALE)
                    if diag_j >= 0:
                        nc.vector.tensor_mul(p_sb[:, w0:w0 + P], p_sb[:, w0:w0 + P],
                                             tri[:])
                    nc.tensor.matmul(
                        pv_ps[:, w0:CH],
                        vnat[:, i, :],
                        p_sb[:, w0:CH],
                        start=(i == 0), stop=(i == n_kt - 1))

                # ---- transpose O'^T chunk to natural, normalize ----
                ovt_sb = ovpool.tile([H + 1, CH], F32)
                nc.scalar.copy(ovt_sb[:], pv_ps[:])
                for j in range(KPC):
                    kt = tq * KPC + j
                    tr_ps = ps_tr.tile([P, H + 1], F32, tag="tro")
                    nc.tensor.transpose(tr_ps[:], ovt_sb[:, j * P:(j + 1) * P],
                                        ident[0:H + 1, 0:H + 1])
                    nc.vector.tensor_copy(onat[:, kt, :], tr_ps[:])
                    recip = ppool.tile([P, 1], F32, tag="recip")
                    nc.vector.reciprocal(recip[:], onat[:, kt, H:H + 1])
                    nc.vector.tensor_scalar_mul(onat[:, kt, 0:H], onat[:, kt, 0:H],
                                                recip[:])

            nc.sync.dma_start(out_r[:], onat[:, :, 0:H])

    nc.finalize()
    return nc


_NC_CACHE = {}


def _get_nc(dt):
    if dt not in _NC_CACHE:
        _NC_CACHE[dt] = build_nc(dt)
    return _NC_CACHE[dt]


def _np_dt(dt):
    return ml_dtypes.bfloat16 if dt == mybir.dt.bfloat16 else np.float32


def run(x, Wk, Wq, Wv, dt=None, **spmd_kwargs):
    dt = dt or MM_DTYPE
    nc = _get_nc(dt)
    ndt = _np_dt(dt)
    wqk = np.ascontiguousarray(
        np.concatenate([Wq, Wk], axis=1).astype(ndt))
    wv = np.ascontiguousarray(np.asarray(Wv).astype(ndt))
    in_maps = [
        {"xt": np.ascontiguousarray(np.asarray(x[b]).T.astype(ndt)),
         "wqk": wqk, "wv": wv}
        for b in range(N_CORES)
    ]
    res = run_bass_kernel_spmd(nc, in_maps, list(range(N_CORES)), **spmd_kwargs)
    out = np.stack([res.results[b]["out"] for b in range(N_CORES)], axis=0)
    return out.astype(np.float32), res


def kernel(x, Wk, Wq, Wv):
    out, _ = run(x, Wk, Wq, Wv)
    return out
